# revision 1
# baseline (speedup 1.0000x reference)
"""GrwSmoothingLoss on 8 Trainium2 NeuronCores.

Math: for each batch b, with Gram matrix G_b = Z_b @ Z_b^T (8x8),
  logits[b,p] = -0.5 * ||diff2(Z_b[perm_p])||^2 = -0.5 * <C_p, G_b>,
  where C_p = M_p^T M_p and M_p is the 6x8 permuted second-difference matrix.
The smoothness term is also linear in G_b:  V_b = 0.5 * <C1, G_b>.
So each core computes, for its 32 batches: G (via elementwise pair products +
reduce), the 64 x 1025 coefficient matrix [ -0.5*C_p | 0.5*alpha*C1 ] (built
on-device from perm_index), one matmul, a logsumexp, and a partition-sum.
Host sums the 8 per-core partials and divides by B.

Sharding: data-parallel over B (32 batches/core); perm_index replicated.
"""

import numpy as np

import concourse.bacc as bacc
import concourse.bass as bass
import concourse.mybir as mybir
import concourse.tile as tile
from concourse.bass_utils import run_bass_kernel_spmd

B, T, K = 256, 8, 128
NUM_PERMS = 1000
PPAD = 1024
ALPHA = 0.5
N_CORES = 8
B_LOC = B // N_CORES
F32 = mybir.dt.float32

_cache = {}


def _consts():
    iota8 = np.broadcast_to(np.arange(8, dtype=np.float32), (128, 8)).copy()
    ident = np.eye(128, dtype=np.float32)
    D1 = (np.eye(T, k=1) - np.eye(T))[: T - 1]
    C1 = D1.T @ D1
    c1col = (0.5 * ALPHA * C1.reshape(T * T, 1)).astype(np.float32)
    ones32 = np.ones((B_LOC, 1), np.float32)
    # q4[(b*4+q), b'] = 1 iff b'==b : folds the 4-way k-split reduction into
    # the Gram transpose matmul
    q4 = np.repeat(np.eye(B_LOC, dtype=np.float32), 4, axis=0)
    return iota8, ident, c1col, ones32, q4


def _kernel_body(tc, out_part, zb_d, permf_d, iota8_d, ident_d, c1col_d, ones32_d, q4_d):
    nc = tc.nc
    P = NUM_PERMS
    with (
        tc.tile_pool(name="sb", bufs=1) as sb,
        tc.tile_pool(name="ps", bufs=1, space="PSUM") as ps,
    ):
        zb = sb.tile([128, 256], F32)
        permf = sb.tile([128, 64], F32)
        iota8 = sb.tile([128, 8], F32)
        ident = sb.tile([128, 128], F32)
        ones32 = sb.tile([B_LOC, 1], F32)
        q4 = sb.tile([128, B_LOC], F32)
        nc.sync.dma_start(out=zb[:], in_=zb_d[:])
        nc.sync.dma_start(out=q4[:], in_=q4_d[:])
        nc.sync.dma_start(out=permf[:], in_=permf_d[:])
        nc.sync.dma_start(out=iota8[:], in_=iota8_d[:])
        nc.sync.dma_start(out=ident[:], in_=ident_d[:])
        nc.sync.dma_start(out=ones32[:], in_=ones32_d[:])

        # one-hot E[(p_lo),(c,t,u)] = (perm[c*128+p_lo, t] == u)
        E = sb.tile([128, 512], F32)
        Ev = E[:].rearrange("p (c t u) -> p c t u", t=8, u=8)
        in0 = (
            permf[:]
            .rearrange("p (c t) -> p c t", t=8)
            .unsqueeze(3)
            .broadcast_to([128, 8, 8, 8])
        )
        in1 = iota8[:].unsqueeze(1).unsqueeze(1).broadcast_to([128, 8, 8, 8])
        nc.vector.tensor_tensor(out=Ev, in0=in0, in1=in1, op=mybir.AluOpType.is_equal)

        # second-difference rows: M[(p_lo),(c,r,u)] = E[.,r,.] - 2E[.,r+1,.] + E[.,r+2,.]
        t1 = sb.tile([128, 384], F32)
        t1v = t1[:].rearrange("p (c r u) -> p c r u", r=6, u=8)
        nc.vector.tensor_tensor(
            out=t1v, in0=Ev[:, :, 0:6, :], in1=Ev[:, :, 2:8, :], op=mybir.AluOpType.add
        )
        M = sb.tile([128, 384], F32)
        Mv = M[:].rearrange("p (c r u) -> p c r u", r=6, u=8)
        nc.vector.scalar_tensor_tensor(
            out=Mv,
            in0=Ev[:, :, 1:7, :],
            scalar=-2.0,
            in1=t1v,
            op0=mybir.AluOpType.mult,
            op1=mybir.AluOpType.add,
        )

        # C_p entries: call_pT[(p_lo),(c,i,j)] = sum_r M[p,c,r,i]*M[p,c,r,j]
        # ISA limit: <=3 free dims per op, so multiply with (cr,i,j) then
        # reduce with a (c,ij,r) view (r innermost -> X-axis reduce).
        prod = sb.tile([128, 3072], F32)
        prodv = prod[:].rearrange("p (cr i j) -> p cr i j", i=8, j=8)
        m_cr = Mv.rearrange("p c r u -> p (c r) u")
        mi = m_cr.unsqueeze(3).broadcast_to([128, 48, 8, 8])
        mj = m_cr.unsqueeze(2).broadcast_to([128, 48, 8, 8])
        nc.vector.tensor_tensor(out=prodv, in0=mi, in1=mj, op=mybir.AluOpType.mult)
        call_pT = sb.tile([128, 512], F32)
        prod_red = prod[:].rearrange("p (c r ij) -> p c ij r", r=6, ij=64)
        nc.vector.reduce_sum(
            out=call_pT[:], in_=prod_red, axis=mybir.AxisListType.X
        )
        # transpose perm-chunks to [64, 1024] and append the C1 column
        psum_call = ps.tile([64, PPAD], F32)
        for c in range(8):
            nc.tensor.transpose(
                psum_call[:, c * 128 : (c + 1) * 128],
                call_pT[:, c * 64 : (c + 1) * 64],
                ident[:],
            )
        callT = sb.tile([64, PPAD + 1], F32)
        nc.scalar.copy(callT[:, 0:PPAD], psum_call[:])
        nc.sync.dma_start(out=callT[:, PPAD : PPAD + 1], in_=c1col_d[:])

        # Gram, k-split 4 ways so all 128 partitions work:
        # row (b*4+q) of zb holds Z[b, :, q*32:(q+1)*32];
        # gq[(b,q),(i,j)] = sum_k' Z[b,i,qk']Z[b,j,qk'] ; the q-sum and the
        # transpose to [ij, b] happen together in the q4 matmul.
        pp4 = sb.tile([128, 2048], F32)
        pp4v = pp4[:].rearrange("p (i j k) -> p i j k", i=8, j=8)
        zv = zb[:].rearrange("p (t k) -> p t k", t=8)
        nc.gpsimd.tensor_tensor(
            out=pp4v,
            in0=zv.unsqueeze(2).broadcast_to([128, 8, 8, 32]),
            in1=zv.unsqueeze(1).broadcast_to([128, 8, 8, 32]),
            op=mybir.AluOpType.mult,
        )
        gq = sb.tile([128, 64], F32)
        nc.vector.reduce_sum(
            out=gq[:],
            in_=pp4v.rearrange("p i j k -> p (i j) k"),
            axis=mybir.AxisListType.X,
        )
        psum_g = ps.tile([64, B_LOC], F32)
        nc.tensor.matmul(psum_g[:], gq[:], q4[:])
        gT = sb.tile([64, B_LOC], F32)
        nc.scalar.copy(gT[:], psum_g[:])

        # X[b, p] = logits ; X[b, PPAD] = alpha * V_b
        psum_X = ps.tile([B_LOC, PPAD + 1], F32)
        nc.tensor.matmul(psum_X[:, 0:512], gT[:], callT[:, 0:512])
        nc.tensor.matmul(psum_X[:, 512:1024], gT[:], callT[:, 512:1024])
        nc.tensor.matmul(psum_X[:, 1024:1025], gT[:], callT[:, 1024:1025])

        # X holds <G, C_p> (unscaled); logits = -0.5*X, so max logit = min X.
        # lse = ln(sum exp(-0.5 X + 0.5 mn)) - 0.5 mn ; Exp's accum_out gives
        # the sum for free.
        mn = sb.tile([B_LOC, 1], F32)
        nc.vector.tensor_reduce(
            out=mn[:], in_=psum_X[:, 0:P], axis=mybir.AxisListType.X,
            op=mybir.AluOpType.min,
        )
        pbias = sb.tile([B_LOC, 1], F32)
        nc.vector.tensor_scalar_mul(pbias[:], mn[:], 0.5)
        e = sb.tile([B_LOC, P], F32)
        s = sb.tile([B_LOC, 1], F32)
        nc.scalar.activation(
            e[:], psum_X[:, 0:P], mybir.ActivationFunctionType.Exp,
            bias=pbias[:], scale=-0.5, accum_out=s[:],
        )
        lns = sb.tile([B_LOC, 1], F32)
        nc.scalar.activation(lns[:], s[:], mybir.ActivationFunctionType.Ln)
        # loss_b = (0.5*X0 + lns) + (alpha*V - 0.5*mn); one PSUM source per op
        u1 = sb.tile([B_LOC, 1], F32)
        nc.vector.scalar_tensor_tensor(
            out=u1[:],
            in0=psum_X[:, 0:1],
            scalar=0.5,
            in1=lns[:],
            op0=mybir.AluOpType.mult,
            op1=mybir.AluOpType.add,
        )
        u2 = sb.tile([B_LOC, 1], F32)
        nc.vector.tensor_tensor(
            out=u2[:], in0=psum_X[:, PPAD : PPAD + 1], in1=pbias[:],
            op=mybir.AluOpType.subtract,
        )
        loss_col = sb.tile([B_LOC, 1], F32)
        nc.vector.tensor_tensor(
            out=loss_col[:], in0=u1[:], in1=u2[:], op=mybir.AluOpType.add
        )

        psum_t = ps.tile([1, 1], F32)
        nc.tensor.matmul(psum_t[:], loss_col[:], ones32[:])
        out_sb = sb.tile([1, 1], F32)
        nc.vector.tensor_copy(out_sb[:], psum_t[:])
        nc.sync.dma_start(out=out_part[:], in_=out_sb[:])


def _build():
    if "nc" in _cache:
        return _cache["nc"]
    nc = bacc.Bacc(
        "TRN2",
        target_bir_lowering=False,
        debug=False,
        enable_asserts=False,
        num_devices=N_CORES,
    )
    zb_d = nc.dram_tensor("zb", [128, 256], F32, kind="ExternalInput").ap()
    permf_d = nc.dram_tensor("permf", [128, 64], F32, kind="ExternalInput").ap()
    iota8_d = nc.dram_tensor("iota8", [128, 8], F32, kind="ExternalInput").ap()
    ident_d = nc.dram_tensor("ident", [128, 128], F32, kind="ExternalInput").ap()
    c1col_d = nc.dram_tensor("c1col", [T * T, 1], F32, kind="ExternalInput").ap()
    ones32_d = nc.dram_tensor("ones32", [B_LOC, 1], F32, kind="ExternalInput").ap()
    q4_d = nc.dram_tensor("q4", [128, B_LOC], F32, kind="ExternalInput").ap()
    out_d = nc.dram_tensor("out_part", [1, 1], F32, kind="ExternalOutput").ap()
    with tile.TileContext(nc) as tc:
        _kernel_body(tc, out_d, zb_d, permf_d, iota8_d, ident_d, c1col_d, ones32_d, q4_d)
    nc.compile()
    _cache["nc"] = nc
    return nc


def _in_maps(Z, perm_index):
    perm = np.asarray(perm_index, dtype=np.int64).reshape(NUM_PERMS, T)
    perm_pad = np.concatenate(
        [perm, np.tile(perm[0:1], (PPAD - NUM_PERMS, 1))], axis=0
    )
    permf = (
        perm_pad.reshape(8, 128, T).transpose(1, 0, 2).reshape(128, 64)
    ).astype(np.float32)
    iota8, ident, c1col, ones32, q4 = _consts()
    Zf = np.asarray(Z, dtype=np.float32).reshape(B, T, 4, 32)
    in_maps = []
    for c in range(N_CORES):
        zb4 = np.ascontiguousarray(
            Zf[c * B_LOC : (c + 1) * B_LOC].transpose(0, 2, 1, 3)
        ).reshape(128, 256)
        in_maps.append(
            {
                "zb": zb4,
                "permf": permf,
                "iota8": iota8,
                "ident": ident,
                "c1col": c1col,
                "ones32": ones32,
                "q4": q4,
            }
        )
    return in_maps


def kernel(Z, perm_index, _trace=False):
    nc = _build()
    in_maps = _in_maps(Z, perm_index)
    res = run_bass_kernel_spmd(
        nc, in_maps, core_ids=list(range(N_CORES)), trace=_trace
    )
    total = np.float64(0.0)
    for r in res.results:
        total += np.float64(r["out_part"][0, 0])
    out = np.array(total / B, dtype=np.float32)
    if _trace:
        return out, res
    return out

